# revision 34
# baseline (speedup 1.0000x reference)
"""ContextBranch (context-RoI pooling + 1x1-conv fusion) on 8 Trainium2 cores.

Problem: for each of N=128 boxes, pool the 8 surrounding context cells
(3x3 grid minus center) from a [256, 64, 64] feature map with ROIAlignV2
(7x7 output, sampling_ratio 2), concatenate the 8 pooled chunks into 2048
channels, apply a 1x1 conv (2048->256) + bias + ReLU.

Sharding: box-parallel. Core m handles boxes [16m, 16m+16) and the 128
context cells they consume. The fusion weights are replicated.

Device algorithm (per core), interp-first with a transposed host gather:
  1. ROIAlign collapses to pooled[b,s,:] = sum_p M_b[s,p] * Wnd_b[p,:]
     with M_b = By (x) Bx in [49, 64] and Wnd_b an 8x8 feature window
     (computed/gathered on host; the walrus build here cannot compile the
     GPSIMD library reload that the on-chip DMAGatherAnt needs).
  2. The gather lands TRANSPOSED: G^T[(b01,iy,ix), (kc,c)] per box pair,
     so windows sit on partitions and are only ever used as the matmul
     STATIONARY operand (Ldweights are free).
  3. Interp GEMM per (pair, kc, b01, c_half):
       pooledT[c128, 49] = G^T[64pix, c128]^T @ M[64pix, 49]
     streaming only 49 rows per matmul; 4 quadrants land in one PSUM
     tile [128, (c_hi, b01, 49)], one engine copy -> SBUF bf16
     (copies rotate over DVE/Pool/Act so no engine saturates).
  4. Fusion GEMM per (box, o_half): out[o128, 49] accumulates 16 matmuls
     lhsT=w[(kc,c_hi,o_hi)][c128, o128], rhs=pooledT[c128, 49] in PSUM.
  5. Bias+ReLU on the scalar engine (bias is per-partition in this
     orientation); one DMA per pair writes [128, (b01,o_hi,49)] fp32.
Host reassembles per-core [8,128,2,2,49] -> [128, 256, 7, 7].
"""

import numpy as np
import ml_dtypes

import concourse.bass as bass
import concourse.tile as tile
from concourse import mybir
from concourse import bass_utils
from concourse.vector_clock import ScopedClock

# ---------------------------------------------------------------- constants
OUT = 7          # output size
SR = 2           # sampling ratio
SCALE = 1.0 / 16.0
H = W = 64
C = 256
N_BOXES = 128
N_CORES = 8
NB = N_BOXES // N_CORES   # 16 boxes per core
K8 = 8                    # context offsets
NPAIR = NB // 2           # 8 box pairs per core
WIN = 8                   # window side
WPX = WIN * WIN           # 64 window pixels
S49 = OUT * OUT           # 49 pooled positions
GCOL = K8 * C             # 2048 gather columns per pair chunk

BF16 = ml_dtypes.bfloat16


# ------------------------------------------------------- tile drain patch
def _patched_drain_and_barrier(self, tick_clock, wait_clock):
    # The walrus build in this environment rejects >1 sync wait on a Drain
    # ("Too many sync wait commands"), but Tile's kernel-tail drain carries
    # one wait per live semaphore. Split into chained single-wait drains on
    # the same engine, which is semantically identical.
    nc = self.nc
    drain_bi = nc.sync.drain()
    inst = drain_bi.ins
    wait_clock.add_sem_waits(inst, ScopedClock({None: tick_clock.global_clock}))
    si = inst.sync_info
    waits = list(si.on_wait) if si is not None else []
    if len(waits) > 1:
        inst.sync_info = mybir.SyncInfo(on_wait=[waits[0]], on_update=[])
        for w in waits[1:]:
            d2 = nc.sync.drain()
            d2.ins.sync_info = mybir.SyncInfo(on_wait=[w], on_update=[])

    nc.all_engine_barrier()
    assert self.sems is not None
    popped = nc._tile_sem_poison_stack.pop()
    assert popped is self._sem_poison
    nc.clear_and_free_semaphores(list(self.sems.allocated().values()))
    nc.all_engine_barrier()


tile.TileContext._drain_and_barrier = _patched_drain_and_barrier

_ws_counter = [0]


def _split_multi_waits(nc):
    """Walrus here allows only ONE sync wait per instruction. For every
    instruction carrying N>1 waits, hoist N-1 of them onto injected NoOps on
    the same engine immediately before it (same-engine program order makes
    this semantically identical)."""
    for f in nc.m.functions:
        for blk in f.blocks:
            new_insts = []
            for inst in blk.instructions:
                si = getattr(inst, "sync_info", None)
                waits = list(si.on_wait) if si is not None else []
                if len(waits) > 1:
                    for w in waits[:-1]:
                        _ws_counter[0] += 1
                        nop = mybir.InstNoOp(
                            name=f"I-waitsplit-{_ws_counter[0]}", ins=[], outs=[]
                        )
                        nop.engine = inst.engine
                        nop.sync_info = mybir.SyncInfo(on_wait=[w], on_update=[])
                        nc.register_instruction(nop)
                        new_insts.append(nop)
                    inst.sync_info = mybir.SyncInfo(
                        on_wait=[waits[-1]], on_update=list(si.on_update)
                    )
                new_insts.append(inst)
            blk.instructions = new_insts


# ------------------------------------------------------------- host math
def _context_boxes(boxes):
    """[N,4] -> [8, N, 4] context cells, offset-major (reference order)."""
    boxes = boxes.astype(np.float32)
    x1, y1, x2, y2 = boxes[:, 0], boxes[:, 1], boxes[:, 2], boxes[:, 3]
    w = (x2 - x1) / np.float32(3.0)
    h = (y2 - y1) / np.float32(3.0)
    offs = []
    for i in range(3):
        for j in range(3):
            if i == 1 and j == 1:
                continue
            dx = j * w
            dy = i * h
            offs.append(np.stack([x1 + dx, y1 + dy, x1 + dx + w, y1 + dy + h], axis=1))
    return np.stack(offs, axis=0)


def _axis_weights(lo_c, hi_c, size):
    """Per-axis pooled interp weights for one axis of all B context boxes.

    lo_c, hi_c: [B] box edge coords (image space). Returns (orig [B] int,
    Wax [B, 7, 8] fp32) with pooling (x0.5) folded in.
    """
    B = lo_c.shape[0]
    start = lo_c * np.float32(SCALE) - np.float32(0.5)
    end = hi_c * np.float32(SCALE) - np.float32(0.5)
    bw = (end - start) / np.float32(OUT)
    j = np.arange(OUT * SR)
    t = (j // SR + ((j % SR) + np.float32(0.5)) / np.float32(SR)).astype(np.float32)
    pos = start[:, None] + t[None, :] * bw[:, None]          # [B, 14]
    valid = (pos >= -1.0) & (pos <= size)
    pc = np.clip(pos, np.float32(0.0), np.float32(size - 1))
    lo = np.clip(np.floor(pc), 0.0, size - 2).astype(np.int64)
    f = (pc - lo.astype(np.float32)).astype(np.float32)
    wl = ((1.0 - f) * valid).astype(np.float32)
    wh = (f * valid).astype(np.float32)
    orig = np.clip(lo.min(axis=1), 0, size - WIN)            # [B]
    rel = lo - orig[:, None]                                 # [B, 14] in [0, 6]
    assert rel.min() >= 0 and rel.max() <= WIN - 2
    Wax = np.zeros((B, OUT, WIN), np.float32)
    bi = np.arange(B)
    for jj in range(OUT * SR):
        g = jj // SR
        Wax[bi, g, rel[:, jj]] += 0.5 * wl[:, jj]
        Wax[bi, g, rel[:, jj] + 1] += 0.5 * wh[:, jj]
    return orig, Wax


def _prep(features, boxes, w_fuse, b_fuse):
    """All host-side layout/index prep. Returns (shared dict, per-core list)."""
    features = np.asarray(features, np.float32)
    boxes = np.asarray(boxes, np.float32)
    w_fuse = np.asarray(w_fuse, np.float32)
    b_fuse = np.asarray(b_fuse, np.float32)

    cb = _context_boxes(boxes).reshape(K8 * N_BOXES, 4)      # [1024, 4]
    B = cb.shape[0]
    ox, Wx = _axis_weights(cb[:, 0], cb[:, 2], W)            # x axis
    oy, Wy = _axis_weights(cb[:, 1], cb[:, 3], H)            # y axis

    # M^T[b, p=(iy,ix), s=(ph,pw)] = Wy[b,ph,iy] * Wx[b,pw,ix]
    MT = (Wy[:, :, None, :, None] * Wx[:, None, :, None, :]) \
        .transpose(0, 3, 4, 1, 2).reshape(B, WPX, S49)

    # gather pixel index of window pixel p=(iy,ix) of cbox b
    iy, ix = np.meshgrid(np.arange(WIN), np.arange(WIN), indexing="ij")
    pix = ((oy[:, None, None] + iy) * W + (ox[:, None, None] + ix)).reshape(B, WPX)
    assert pix.min() >= 0 and pix.max() < H * W

    # shared tensors
    # w_sb[c_lo, (kc, c_hi, o_hi, o_lo)] = w_fuse[o_hi*128+o_lo, kc*256+c_hi*128+c_lo]
    w5 = w_fuse.reshape(2, 128, K8, 2, 128)                  # [o_hi, o_lo, kc, c_hi, c_lo]
    wsb = np.ascontiguousarray(
        w5.transpose(4, 2, 3, 0, 1).reshape(128, K8 * 2 * 2 * 128)
    ).astype(BF16)
    bo = np.concatenate([b_fuse.reshape(1, C),
                         np.ones((1, S49), np.float32)], axis=1).astype(BF16)
    shared = {"wsb": wsb, "bo": bo}

    # The reference reshapes offset-major pooled [8N,...] to [N, 2048, ...]:
    # output box n is fused from cboxes 8n+kc (kc = chunk group 0..7).
    featT = np.ascontiguousarray(features.reshape(C, H * W).T).astype(BF16)

    per_core = []
    for m in range(N_CORES):
        # cbox ids per (np, b01, kc)
        n_ids = NB * m + 2 * np.arange(NPAIR)[:, None, None] + np.arange(2)[None, :, None]
        cb_ids = 8 * n_ids + np.arange(K8)[None, None, :]    # [8, 2, 8]

        # mbd[(b01, pix64), np, kc*49] = MT[cbox]
        mcore = MT[cb_ids]                                   # [np, b01, kc, 64, 49]
        mbd = mcore.transpose(1, 3, 0, 2, 4).reshape(128, NPAIR, K8 * S49)

        # G^T[(b01, pix64), np, (kc, c)] = feat[c, pix[cbox][p]]
        g = featT[pix[cb_ids]].astype(np.float32)            # [np, b01, kc, 64, 256]
        g = g.transpose(1, 3, 0, 2, 4).reshape(128, NPAIR, GCOL)
        # one fused [G | M] block per pair so each pair arrives in ONE DMA;
        # pair 0 is kh-blocked ([Gkh0|Mkh0|Gkh1|Mkh1]) so its first half can
        # land in a separate, earlier DMA
        gm = np.concatenate([g, mbd], axis=2).astype(BF16)   # [128, np, 2440]
        gm[:, 0, :] = np.concatenate(
            [g[:, 0, 0:1024], mbd[:, 0, 0:196],
             g[:, 0, 1024:2048], mbd[:, 0, 196:392]], axis=1).astype(BF16)
        gmsh = np.ascontiguousarray(gm.reshape(128, NPAIR * (GCOL + K8 * S49)))
        per_core.append({"gmsh": gmsh})
    return shared, per_core


# ------------------------------------------------------------ device build
def _build_nc():
    nc = bass.Bass("TRN2", target_bir_lowering=False, debug=False,
                   num_devices=N_CORES, dynamic_dma_scratch_size=32768)
    dt = mybir.dt
    GMW = GCOL + K8 * S49
    wsb = nc.dram_tensor("wsb", [128, K8 * 2 * 2 * 128], dt.bfloat16, kind="ExternalInput").ap()
    bo = nc.dram_tensor("bo", [1, C + S49], dt.bfloat16, kind="ExternalInput").ap()
    gmsh = nc.dram_tensor("gmsh", [128, NPAIR * GMW], dt.bfloat16, kind="ExternalInput").ap()
    out = nc.dram_tensor("out", [NPAIR, 128, 2, 2, S49], dt.float32, kind="ExternalOutput").ap()

    with tile.TileContext(nc) as tc:
        with (
            tc.tile_pool(name="const", bufs=1) as const,
            tc.tile_pool(name="g", bufs=NPAIR) as gpool,
            tc.tile_pool(name="psb", bufs=12) as psb_pool,
            tc.tile_pool(name="pps", bufs=6, space="PSUM") as pps_pool,
            tc.tile_pool(name="ops", bufs=2, space="PSUM") as ops_pool,
            tc.tile_pool(name="osb", bufs=3) as osb_pool,
        ):
            # DMA issue order: (mbd, G) chunk for pair 0 first so the PE
            # can start at ~4us; the two w halves land before fusion(0)
            # needs each o_hi; later pairs stream in behind.
            g_tiles = []
            for np_ in range(NPAIR):
                g_sb = gpool.tile([128, GMW], dt.bfloat16)
                g_tiles.append(g_sb)
            w_sb = const.tile([128, K8 * 2 * 2 * 128], dt.bfloat16)
            bo_sb = const.tile([1, C + S49], dt.bfloat16)

            def chunk(np_):
                nc.sync.dma_start(g_tiles[np_][:], gmsh[:, np_ * GMW:(np_ + 1) * GMW])

            nc.sync.dma_start(g_tiles[0][:, 0:GMW // 2], gmsh[:, 0:GMW // 2])
            nc.sync.dma_start(g_tiles[0][:, GMW // 2:GMW],
                              gmsh[:, GMW // 2:GMW])
            nc.sync.dma_start(w_sb[:, 0:1024], wsb[:, 0:1024])
            chunk(1)
            nc.sync.dma_start(w_sb[:, 1024:2048], wsb[:, 1024:2048])
            chunk(2)
            nc.sync.dma_start(w_sb[:, 2048:3072], wsb[:, 2048:3072])
            nc.sync.dma_start(bo_sb[:], bo[:])
            chunk(3)
            nc.sync.dma_start(w_sb[:, 3072:4096], wsb[:, 3072:4096])
            for np_ in range(4, NPAIR):
                chunk(np_)

            # PSUM->SBUF copies: GPSIMD cannot read PSUM on this HW, so
            # rotate over DVE (3/4) and Act (1/4).
            copy_engines = [nc.vector, nc.scalar, nc.scalar, nc.vector]

            def interp(np_):
                """Interp GEMMs + PSUM->SBUF copies for pair np_; returns 4
                bf16 pooledT tiles keyed (kh, b01).

                Each (kh, b01) has its own PSUM bank: the walrus runtime
                miscompiles matmul sequences that mix input partition ranges
                within one PSUM bank, so part-0:64 and part-64:128 matmuls
                never share a bank."""
                g_sb = g_tiles[np_]
                tiles = []
                for kh in range(2):
                    for b01 in range(2):
                        p_ps = pps_pool.tile([128, 4, 2, S49], dt.float32)
                        for kl in range(4):
                            kc = 4 * kh + kl
                            if np_ == 0:   # kh-blocked layout (see _prep)
                                goff = kh * 1220 + kl * C
                                moff = kh * 1220 + 1024 + kl * S49
                            else:
                                goff = kc * C
                                moff = GCOL + kc * S49
                            for c_hi in range(2):
                                nc.tensor.matmul(
                                    p_ps[:, kl, c_hi, :],
                                    lhsT=g_sb[64 * b01:64 * (b01 + 1),
                                              goff + c_hi * 128:goff + (c_hi + 1) * 128],
                                    rhs=g_sb[64 * b01:64 * (b01 + 1),
                                             moff:moff + S49],
                                    start=True, stop=True,
                                )
                        p_sb = psb_pool.tile([128, 4, 2, S49], dt.bfloat16)
                        eng = copy_engines[(kh * 2 + b01) % len(copy_engines)]
                        if eng is nc.scalar:
                            eng.activation(p_sb[:], p_ps[:],
                                           mybir.ActivationFunctionType.Copy)
                        else:
                            eng.tensor_copy(p_sb[:], p_ps[:])
                        tiles.append(p_sb)
                return tiles

            def fuse(np_, p_tiles):
                """Fusion GEMMs (sequential PSUM groups per quadrant), rank-1
                bias, ReLU, out DMA for pair np_. The last pair splits ReLU +
                out DMA per box half to shorten the kernel tail."""
                last = np_ == NPAIR - 1
                o_ps = ops_pool.tile([128, 2, 2, S49], dt.float32)
                o_sb = osb_pool.tile([128, 2, 2, S49], dt.float32)
                quads = ([(0, 0), (0, 1), (1, 0), (1, 1)] if last
                         else [(0, 0), (1, 0), (0, 1), (1, 1)])
                for b01, o_hi in quads:
                    for kc in range(K8):
                        for c_hi in range(2):
                            nc.tensor.matmul(
                                o_ps[:, b01, o_hi, :],
                                lhsT=w_sb[:, ((kc * 2 + c_hi) * 2 + o_hi) * 128:
                                          ((kc * 2 + c_hi) * 2 + o_hi + 1) * 128],
                                rhs=p_tiles[(kc // 4) * 2 + b01][:, kc % 4, c_hi, :],
                                start=(kc == 0 and c_hi == 0),
                                stop=False,
                            )
                    nc.tensor.matmul(
                        o_ps[:, b01, o_hi, :],
                        lhsT=bo_sb[0:1, o_hi * 128:(o_hi + 1) * 128],
                        rhs=bo_sb[0:1, C:C + S49],
                        start=False, stop=True,
                    )
                    if last and o_hi == 1:
                        nc.scalar.activation(
                            o_sb[:, b01, :, :].rearrange("p a b -> p (a b)"),
                            o_ps[:, b01, :, :].rearrange("p a b -> p (a b)"),
                            mybir.ActivationFunctionType.Relu,
                        )
                        dst = out[np_, :, b01, :, :].rearrange("p a b -> p (a b)")
                        nc.sync.dma_start(
                            dst, o_sb[:, b01, :, :].rearrange("p a b -> p (a b)"))
                if not last:
                    nc.scalar.activation(
                        o_sb[:].rearrange("p a b c -> p (a b c)"),
                        o_ps[:].rearrange("p a b c -> p (a b c)"),
                        mybir.ActivationFunctionType.Relu,
                    )
                    dst = out[np_, :, :, :, :].rearrange("p a b c -> p (a b c)")
                    nc.sync.dma_start(dst, o_sb[:].rearrange("p a b c -> p (a b c)"))

            # software pipeline, depth 2: fusion(np) runs two interp
            # batches after interp(np), so PSUM->SBUF copies never sit on
            # the PE critical path.
            pending = {}
            for np_ in range(NPAIR):
                pending[np_] = interp(np_)
                if np_ >= 2:
                    fuse(np_ - 2, pending.pop(np_ - 2))
            fuse(NPAIR - 2, pending.pop(NPAIR - 2))
            fuse(NPAIR - 1, pending.pop(NPAIR - 1))
    _split_multi_waits(nc)
    return nc


_NC_CACHE = None


def _get_nc():
    global _NC_CACHE
    if _NC_CACHE is None:
        _NC_CACHE = _build_nc()
    return _NC_CACHE


def make_in_maps(features, boxes, w_fuse, b_fuse):
    shared, per_core = _prep(features, boxes, w_fuse, b_fuse)
    return [{**shared, **pc} for pc in per_core]


def kernel(features, boxes, w_fuse, b_fuse):
    in_maps = make_in_maps(features, boxes, w_fuse, b_fuse)
    nc = _get_nc()
    res = bass_utils.run_bass_kernel_spmd(
        nc, in_maps, core_ids=list(range(N_CORES)), trace=False
    )
    parts = []
    for m in range(N_CORES):
        o = res.results[m]["out"]                 # [8, 128, 2, 2, 49]
        o = o.transpose(0, 2, 3, 1, 4).reshape(NB, C, S49)
        parts.append(o)
    full = np.concatenate(parts, axis=0)          # [128, 256, 49]
    out = full.reshape(N_BOXES, C, OUT, OUT)
    return np.ascontiguousarray(out.astype(np.float32))
